# revision 3
# baseline (speedup 1.0000x reference)
"""Two-layer GRU + residual on 8 Trainium2 NeuronCores — v3.

Strategy (v2): sequence-chunked streams as in v1, but with
  * h-stationary matmuls: per contraction chunk c, lhsT = h[:, c, :]
    ([hid-c, rows]) is the stationary operand and the rhs is a 512-wide
    slice of the weight matrix, so each gate bank needs 4 matmuls of
    N=512 instead of 16 of N=128.  Gate psums come out [rows, gates].
  * per-tick PE transpose of the new state back to [hid, rows] (4
    transposes of 128x128 into one PSUM bank), which simultaneously
    produces the bf16 y-history slot used by layer 2.
  * two-level warmup: layer 1 warms up W1 ticks and emits W2+L outputs;
    layer 2 consumes the first W2 as its own warmup.  W1=W2=16 (state
    influence decays ~0.55/step; 16 steps => ~2e-3, well under the 2e-2
    gate).  v1 used W=64.
  * y history lives entirely in SBUF (no DRAM roundtrip between layers).
Ticks: 96 (L1) + 80 (L2) vs v1's 256, with ~3x fewer PE instructions
per tick.  v3 additionally precomputes layer-1's input GEMM on the host
(f32 BLAS, biases folded) and injects it into PSUM with one identity
matmul per gate, dropping 15 of 32 matmuls per layer-1 tick.
"""

import sys
import numpy as np
import ml_dtypes

sys.path.insert(0, "/opt/trn_rl_repo")

# ---- problem constants (hardcoded per contract) ----
B, T, IN, H = 16, 4096, 512, 512
NCORES = 8
S = 8             # streams (time chunks) per core
R = S * B         # 128 rows per core
L = 64            # useful chunk length; NCORES*S*L == T
W1 = 16           # layer-1 warmup ticks
W2 = 16           # layer-2 warmup ticks
TK1 = W1 + W2 + L # 96 layer-1 ticks
TK2 = W2 + L      # 80 layer-2 ticks
C = 4             # hidden chunks of 128 (H/128)
GW = 512          # gate width per psum bank (H)
SLAB = 4          # ticks per layer-1 xg DMA slab

_cache = {}


def _build_bass():
    import concourse.bass as bass
    import concourse.tile as tile
    from concourse import mybir

    f32 = mybir.dt.float32
    bf16 = mybir.dt.bfloat16
    SIG = mybir.ActivationFunctionType.Sigmoid
    TANH = mybir.ActivationFunctionType.Tanh
    COPY = mybir.ActivationFunctionType.Copy

    nc = bass.Bass("TRN2")

    # layer-1 input gates precomputed on host (f32 GEMM, bf16 shipped):
    # [rows, tick, 3H] with b_ih1 + (b_hh1 for r,z) folded in
    xgd = nc.dram_tensor("xgd", [128, TK1, 3 * H], bf16, kind="ExternalInput")
    wih = [None, None,
           nc.dram_tensor("wih2", [128, C, 3 * H], bf16, kind="ExternalInput")]
    whh = [None, nc.dram_tensor("whh1", [128, C, 3 * H], bf16, kind="ExternalInput"),
           nc.dram_tensor("whh2", [128, C, 3 * H], bf16, kind="ExternalInput")]
    # per-gate bias rows (r, z, xn, hn) on partition 0, each GW wide
    biasd = [None, nc.dram_tensor("bias1", [128, 4, GW], bf16, kind="ExternalInput"),
             nc.dram_tensor("bias2", [128, 4, GW], bf16, kind="ExternalInput")]
    onesd = nc.dram_tensor("onesd", [128, 128], bf16, kind="ExternalInput")
    identd = nc.dram_tensor("identd", [128, 128], bf16, kind="ExternalInput")
    maskd = nc.dram_tensor("maskd", [128, 1], f32, kind="ExternalInput")
    od = nc.dram_tensor("od", [128, C, L, R], f32, kind="ExternalOutput")

    with tile.TileContext(nc) as tc:
        with (
            tc.tile_pool(name="const", bufs=1) as const,
            tc.tile_pool(name="state", bufs=1) as state,
            tc.tile_pool(name="xslab", bufs=2) as xslab,
            tc.tile_pool(name="ew", bufs=2) as ew,
            tc.tile_pool(name="outp", bufs=3) as outp,
            tc.tile_pool(name="pra", bufs=2, space="PSUM") as pra,
            tc.tile_pool(name="pzb", bufs=2, space="PSUM") as pzb,
            tc.tile_pool(name="pxc", bufs=2, space="PSUM") as pxc,
            tc.tile_pool(name="phn", bufs=1, space="PSUM") as phnp,
            tc.tile_pool(name="ptr", bufs=1, space="PSUM") as ptr,
        ):
            # ---- constants to SBUF ----
            wih_sb, whh_sb, bias_sb = {}, {}, {}
            for ell in (1, 2):
                if ell == 2:
                    wih_sb[ell] = const.tile([128, C, 3 * H], bf16, tag=f"wih{ell}", name=f"wih_sb{ell}")
                    nc.sync.dma_start(out=wih_sb[ell], in_=wih[ell][:])
                whh_sb[ell] = const.tile([128, C, 3 * H], bf16, tag=f"whh{ell}", name=f"whh_sb{ell}")
                nc.sync.dma_start(out=whh_sb[ell], in_=whh[ell][:])
                bias_sb[ell] = const.tile([128, 4, GW], bf16, tag=f"bias{ell}", name=f"bias_sb{ell}")
                nc.sync.dma_start(out=bias_sb[ell], in_=biasd[ell][:])
            ones_sb = const.tile([128, 128], bf16)
            nc.sync.dma_start(out=ones_sb, in_=onesd[:])
            ident_sb = const.tile([128, 128], bf16)
            nc.sync.dma_start(out=ident_sb, in_=identd[:])
            mask_sb = const.tile([128, 1], f32)
            nc.sync.dma_start(out=mask_sb, in_=maskd[:])

            # y history: slot k = h1 state BEFORE layer-1 tick k (slot 0 = 0)
            ybf = const.tile([128, C, TK1 + 1, R], bf16, name="ybf")
            h32 = state.tile([128, GW], f32)       # [rows, hid] fp32 state
            hbf2 = state.tile([128, C, R], bf16)   # layer-2 [hid, rows] state

            for ell in (1, 2):
                wi, wh, bi = wih_sb.get(ell), whh_sb[ell], bias_sb[ell]
                TK = TK1 if ell == 1 else TK2
                WM = W1 + W2 if ell == 1 else W2   # mask tick boundary
                nc.vector.memset(h32, 0.0)
                if ell == 1:
                    nc.vector.memset(ybf[:, :, 0, :], 0.0)
                else:
                    nc.vector.memset(hbf2, 0.0)

                xs_cur = None
                ps = [None, None]

                def load_slab(t0):
                    nonlocal xs_cur
                    xs_cur = xslab.tile([128, SLAB, 3 * H], bf16, tag="xs")
                    nc.sync.dma_start(out=xs_cur, in_=xgd[:, t0:t0 + SLAB, :])

                def hlhs(c, tau):
                    if ell == 1:
                        return ybf[:, c, tau, :]
                    return hbf2[:, c, :]

                def prefill(tau):
                    """bias + input-side gates for tick tau (psum dbl-buffered).

                    Layer 1: host-precomputed xg row-block is injected into
                    PSUM via one identity matmul per gate (out = I.T @ xg);
                    biases are folded into xg on the host.
                    Layer 2: bias matmul + y-side GEMM as usual."""
                    ps_r = pra.tile([128, GW], f32, tag="ps_r")
                    ps_z = pzb.tile([128, GW], f32, tag="ps_z")
                    ps_xn = pxc.tile([128, GW], f32, tag="ps_xn")
                    if ell == 1:
                        for gi, p in ((0, ps_r), (1, ps_z), (2, ps_xn)):
                            nc.tensor.matmul(
                                p, ident_sb,
                                xs_cur[:, tau % SLAB, gi * GW:(gi + 1) * GW],
                                start=True, stop=(gi == 2))
                        return [ps_r, ps_z, ps_xn]
                    for gi, p in ((0, ps_r), (1, ps_z), (2, ps_xn)):
                        nc.tensor.matmul(p, ones_sb[0:1, :], bi[0:1, gi, :],
                                         start=True, stop=False)
                    for c in range(C):
                        lx = ybf[:, c, W1 + 1 + tau, :]
                        nc.tensor.matmul(ps_r, lx, wi[:, c, 0:GW],
                                         start=False, stop=False)
                        nc.tensor.matmul(ps_z, lx, wi[:, c, GW:2 * GW],
                                         start=False, stop=False)
                        nc.tensor.matmul(ps_xn, lx, wi[:, c, 2 * GW:3 * GW],
                                         start=False, stop=(c == C - 1))
                    return [ps_r, ps_z, ps_xn]

                for tau in range(TK):
                    if tau == 0:
                        if ell == 1:
                            load_slab(0)
                        ps[0] = prefill(0)
                    ps_r, ps_z, ps_xn = ps[tau % 2]

                    # hidden-side: r first (EW starts early), then hn, then z
                    for c in range(C):
                        nc.tensor.matmul(ps_r, hlhs(c, tau), wh[:, c, 0:GW],
                                         start=False, stop=(c == C - 1))
                    ps_hn = phnp.tile([128, GW], f32, tag="ps_hn")
                    nc.tensor.matmul(ps_hn, ones_sb[0:1, :], bi[0:1, 3, :],
                                     start=True, stop=False)
                    for c in range(C):
                        nc.tensor.matmul(ps_hn, hlhs(c, tau), wh[:, c, 2 * GW:3 * GW],
                                         start=False, stop=(c == C - 1))
                    for c in range(C):
                        nc.tensor.matmul(ps_z, hlhs(c, tau), wh[:, c, GW:2 * GW],
                                         start=False, stop=(c == C - 1))

                    # prefill next tick: runs on PE while DVE/ACT do this tick's EW
                    if tau + 1 < TK:
                        if ell == 1 and (tau + 1) % SLAB == 0:
                            load_slab(tau + 1)
                        ps[(tau + 1) % 2] = prefill(tau + 1)

                    # ---- elementwise ([rows, hid] orientation) ----
                    r_t = ew.tile([128, GW], bf16, tag="r")
                    z_t = ew.tile([128, GW], bf16, tag="z")
                    v_t = ew.tile([128, GW], bf16, tag="v")
                    np_t = ew.tile([128, GW], f32, tag="npre")
                    n_t = ew.tile([128, GW], bf16, tag="n")
                    d_t = ew.tile([128, GW], bf16, tag="d")
                    e_t = ew.tile([128, GW], bf16, tag="e")
                    hrow = ew.tile([128, GW], bf16, tag="hrow")
                    nc.scalar.activation(r_t, ps_r, SIG)
                    nc.vector.tensor_mul(v_t, ps_hn, r_t)       # r*(hn+b_hn)
                    nc.vector.tensor_add(np_t, ps_xn, v_t)
                    nc.scalar.activation(n_t, np_t, TANH)
                    nc.scalar.activation(z_t, ps_z, SIG)
                    nc.vector.tensor_sub(d_t, h32, n_t)
                    nc.vector.tensor_mul(e_t, z_t, d_t)
                    nc.vector.tensor_add(h32, n_t, e_t)         # h' = n + z*(h-n)
                    if tau == WM - 1:
                        # zero rows of stream 0 on core 0 (true h at t=0 is 0)
                        nc.vector.tensor_scalar_mul(h32, h32, mask_sb[:, 0:1])
                    nc.scalar.copy(hrow, h32)                   # bf16 cast

                    # transpose back to [hid, rows]; all 4 chunks in one bank
                    pT = ptr.tile([128, C, R], bf16, tag="pT")
                    for c in range(C):
                        nc.tensor.matmul(pT[:, c, :], hrow[:, c * 128:(c + 1) * 128],
                                         ident_sb, is_transpose=True,
                                         start=(c == 0), stop=(c == C - 1))
                    if ell == 1:
                        nc.scalar.copy(ybf[:, :, tau + 1, :], pT[:, :, :])
                    else:
                        nc.scalar.copy(hbf2, pT[:, :, :])
                        if tau >= W2:
                            ot = outp.tile([128, C, R], f32, tag="ot")
                            nc.vector.tensor_add(ot, pT[:, :, :],
                                                 ybf[:, :, W1 + 1 + tau, :])
                            nc.sync.dma_start(out=od[:, :, tau - W2, :], in_=ot)
    return nc


def _legalize_waits(nc):
    """Hardware instruction encodings hold a limited number of sync waits
    (core_v3 Matmult: 1, DVE STT and friends: 2).  Spill excess waits onto
    same-engine NoOps inserted immediately before the instruction."""
    import bass_rust
    from concourse import mybir

    caps = {}
    nop_cap = 1
    moved = 0
    uid = [0]
    for blk in nc.m.functions[0].blocks:
        idx = 0
        while idx < len(blk.instructions):
            ins = blk.instructions[idx]
            ty = type(ins).__name__
            if ty in ("InstNoOp", "InstEventSemaphore",
                      "InstUnconditionalBranch", "InstCall", "InstISA"):
                idx += 1
                continue
            si = ins.sync_info
            if si is None:
                idx += 1
                continue
            cap = caps.get(ty, 1)
            waits = list(si.on_wait)
            if len(waits) <= cap:
                idx += 1
                continue
            excess = waits[:-cap] if cap else waits
            keep = waits[-cap:] if cap else []
            nops = []
            while excess:
                chunk, excess = excess[:nop_cap], excess[nop_cap:]
                uid[0] += 1
                nop = mybir.InstNoOp(name=f"waitnop-{uid[0]}", ins=[], outs=[])
                nop.engine = ins.engine
                nop.sync_info = bass_rust.SyncInfo(on_wait=chunk, on_update=[])
                nops.append(nop)
                moved += len(chunk)
            for k, nop in enumerate(nops):
                blk.instructions.insert(idx + k, nop)
            ins2 = blk.instructions[idx + len(nops)]
            assert ins2.name == ins.name
            si.on_wait = keep
            ins2.sync_info = si
            idx += len(nops) + 1
    return moved


def _prep_inputs(x, W_ih1, W_hh1, b_ih1, b_hh1, W_ih2, W_hh2, b_ih2, b_hh2):
    bf = ml_dtypes.bfloat16
    f32 = np.float32
    WP = W1 + W2  # x prefix ticks

    def wT(Wm):  # [3H, H] -> [128, C, 3H] tiles: [p, c, g] = Wm[g, c*128+p]
        return np.ascontiguousarray(
            Wm.T.reshape(C, 128, 3 * H).transpose(1, 0, 2)).astype(bf)

    def biasrows(bi, bh):  # per-gate bias rows on partition 0: r, z, xn, hn
        out = np.zeros((128, 4, GW), np.float32)
        s = bi + bh
        out[0, 0, :] = s[:H]
        out[0, 1, :] = s[H:2 * H]
        out[0, 2, :] = bi[2 * H:]
        out[0, 3, :] = bh[2 * H:]
        return out.astype(bf)

    ones = np.zeros((128, 128), np.float32)
    ones[0, :] = 1.0
    ident = np.eye(128, dtype=np.float32)
    common = {
        "whh1": wT(W_hh1),
        "wih2": wT(W_ih2), "whh2": wT(W_hh2),
        "bias1": biasrows(b_ih1, b_hh1), "bias2": biasrows(b_ih2, b_hh2),
        "onesd": ones.astype(bf), "identd": ident.astype(bf),
    }

    # layer-1 input gates on host: f32 BLAS GEMM, biases folded
    # (r,z get b_ih+b_hh; xn gets b_ih only — b_hh_n rides in the hn psum)
    bfold = np.concatenate([(b_ih1 + b_hh1)[:2 * H], b_ih1[2 * H:]]).astype(np.float32)
    xg = x.reshape(-1, IN).astype(np.float32) @ W_ih1.T.astype(np.float32)
    xg = (xg.reshape(B, T, 3 * H) + bfold).astype(np.float32)
    # x is zero-padded for t<0, so xg there is just the bias row
    xgpad = np.concatenate(
        [np.broadcast_to(bfold, (B, WP, 3 * H)), xg], axis=1)  # [B, T+WP, 3H]
    in_maps = []
    for p in range(NCORES):
        segs = np.stack([xgpad[:, (p * S + s) * L: (p * S + s) * L + TK1, :]
                         for s in range(S)])              # [S, B, TK1, 3H]
        xgp = segs.transpose(0, 1, 2, 3).reshape(S * B, TK1, 3 * H).astype(bf)
        mask = np.ones((128, 1), np.float32)
        if p == 0:
            mask[0:B, 0] = 0.0  # rows of stream 0 (true h at chunk start is 0)
        in_maps.append({"xgd": np.ascontiguousarray(xgp),
                        "maskd": mask, **common})
    return in_maps


def _postprocess(results):
    out = np.empty((B, T, H), np.float32)
    for p in range(NCORES):
        o = results[p]["od"]                    # [128, C, L, R]
        o = o.reshape(128, C, L, S, B).transpose(4, 3, 2, 1, 0) \
             .reshape(B, S * L, H)
        out[:, p * S * L:(p + 1) * S * L, :] = o
    return out


def kernel(**inputs):
    from concourse.bass_utils import run_bass_kernel_spmd

    if "nc" not in _cache:
        nc = _build_bass()
        _legalize_waits(nc)
        _cache["nc"] = nc
    nc = _cache["nc"]
    in_maps = _prep_inputs(**inputs)
    res = run_bass_kernel_spmd(nc, in_maps, core_ids=list(range(NCORES)))
    return _postprocess(res.results)


# revision 4
# speedup vs baseline: 1.0155x; 1.0155x over previous
"""Two-layer GRU + residual on 8 Trainium2 NeuronCores — v3.

Strategy: sequence-chunked streams as in v1, but with
  * h-stationary matmuls: per contraction chunk c, lhsT = h[:, c, :]
    ([hid-c, rows]) is the stationary operand and the rhs is a 512-wide
    slice of the weight matrix, so each gate bank needs 4 matmuls of
    N=512 instead of 16 of N=128.  Gate psums come out [rows, gates].
  * per-tick PE transpose of the new state back to [hid, rows] (4
    transposes of 128x128 into one PSUM bank), which simultaneously
    produces the bf16 y-history slot used by layer 2.
  * two-level warmup: layer 1 warms up W1 ticks and emits W2+L outputs;
    layer 2 consumes the first W2 as its own warmup.  W1=W2=16 (state
    influence decays ~0.55/step; 16 steps => ~2e-3, well under the 2e-2
    gate).  v1 used W=64.
  * y history lives entirely in SBUF (no DRAM roundtrip between layers).
Ticks: 96 (L1) + 80 (L2) vs v1's 256, with ~3x fewer PE instructions
per tick.  v3 additionally precomputes layer-1's input GEMM on the host
(f32 BLAS, biases folded) and injects it into PSUM with one identity
matmul per gate, dropping 15 of 32 matmuls per layer-1 tick.
"""

import sys
import numpy as np
import ml_dtypes

sys.path.insert(0, "/opt/trn_rl_repo")

# ---- problem constants (hardcoded per contract) ----
B, T, IN, H = 16, 4096, 512, 512
NCORES = 8
S = 8             # streams (time chunks) per core
R = S * B         # 128 rows per core
L = 64            # useful chunk length; NCORES*S*L == T
W1 = 16           # layer-1 warmup ticks
W2 = 16           # layer-2 warmup ticks
TK1 = W1 + W2 + L # 96 layer-1 ticks
TK2 = W2 + L      # 80 layer-2 ticks
C = 4             # hidden chunks of 128 (H/128)
GW = 512          # gate width per psum bank (H)
SLAB = 4          # ticks per layer-1 xg DMA slab

_cache = {}


def _build_bass():
    import concourse.bass as bass
    import concourse.tile as tile
    from concourse import mybir

    f32 = mybir.dt.float32
    bf16 = mybir.dt.bfloat16
    SIG = mybir.ActivationFunctionType.Sigmoid
    TANH = mybir.ActivationFunctionType.Tanh
    COPY = mybir.ActivationFunctionType.Copy

    nc = bass.Bass("TRN2")

    # layer-1 input gates precomputed on host (f32 GEMM, bf16 shipped):
    # [rows, tick, 3H] with b_ih1 + (b_hh1 for r,z) folded in
    xgd = nc.dram_tensor("xgd", [128, TK1, 3 * H], bf16, kind="ExternalInput")
    wih = [None, None,
           nc.dram_tensor("wih2", [128, C, 3 * H], bf16, kind="ExternalInput")]
    whh = [None, nc.dram_tensor("whh1", [128, C, 3 * H], bf16, kind="ExternalInput"),
           nc.dram_tensor("whh2", [128, C, 3 * H], bf16, kind="ExternalInput")]
    # per-gate bias rows (r, z, xn, hn) on partition 0, each GW wide
    biasd = [None, nc.dram_tensor("bias1", [128, 4, GW], bf16, kind="ExternalInput"),
             nc.dram_tensor("bias2", [128, 4, GW], bf16, kind="ExternalInput")]
    onesd = nc.dram_tensor("onesd", [128, 128], bf16, kind="ExternalInput")
    identd = nc.dram_tensor("identd", [128, 128], bf16, kind="ExternalInput")
    maskd = nc.dram_tensor("maskd", [128, 1], f32, kind="ExternalInput")
    od = nc.dram_tensor("od", [128, C, L, R], f32, kind="ExternalOutput")

    with tile.TileContext(nc) as tc:
        with (
            tc.tile_pool(name="const", bufs=1) as const,
            tc.tile_pool(name="state", bufs=1) as state,
            tc.tile_pool(name="xslab", bufs=2) as xslab,
            tc.tile_pool(name="ew", bufs=2) as ew,
            tc.tile_pool(name="outp", bufs=3) as outp,
            tc.tile_pool(name="pra", bufs=2, space="PSUM") as pra,
            tc.tile_pool(name="pzb", bufs=2, space="PSUM") as pzb,
            tc.tile_pool(name="pxc", bufs=2, space="PSUM") as pxc,
            tc.tile_pool(name="phn", bufs=1, space="PSUM") as phnp,
            tc.tile_pool(name="ptr", bufs=1, space="PSUM") as ptr,
        ):
            # ---- constants to SBUF ----
            wih_sb, whh_sb, bias_sb = {}, {}, {}
            for ell in (1, 2):
                if ell == 2:
                    wih_sb[ell] = const.tile([128, C, 3 * H], bf16, tag=f"wih{ell}", name=f"wih_sb{ell}")
                    nc.sync.dma_start(out=wih_sb[ell], in_=wih[ell][:])
                whh_sb[ell] = const.tile([128, C, 3 * H], bf16, tag=f"whh{ell}", name=f"whh_sb{ell}")
                nc.sync.dma_start(out=whh_sb[ell], in_=whh[ell][:])
                bias_sb[ell] = const.tile([128, 4, GW], bf16, tag=f"bias{ell}", name=f"bias_sb{ell}")
                nc.sync.dma_start(out=bias_sb[ell], in_=biasd[ell][:])
            ones_sb = const.tile([128, 128], bf16)
            nc.sync.dma_start(out=ones_sb, in_=onesd[:])
            ident_sb = const.tile([128, 128], bf16)
            nc.sync.dma_start(out=ident_sb, in_=identd[:])
            mask_sb = const.tile([128, 1], f32)
            nc.sync.dma_start(out=mask_sb, in_=maskd[:])

            # y history: slot k = h1 state BEFORE layer-1 tick k (slot 0 = 0)
            ybf = const.tile([128, C, TK1 + 1, R], bf16, name="ybf")
            h32 = state.tile([128, GW], f32)       # [rows, hid] fp32 state
            hbf2 = state.tile([128, C, R], bf16)   # layer-2 [hid, rows] state

            for ell in (1, 2):
                wi, wh, bi = wih_sb.get(ell), whh_sb[ell], bias_sb[ell]
                TK = TK1 if ell == 1 else TK2
                WM = W1 + W2 if ell == 1 else W2   # mask tick boundary
                nc.vector.memset(h32, 0.0)
                if ell == 1:
                    nc.vector.memset(ybf[:, :, 0, :], 0.0)
                else:
                    nc.vector.memset(hbf2, 0.0)

                xs_cur = None
                ps = [None, None]

                def load_slab(t0):
                    nonlocal xs_cur
                    xs_cur = xslab.tile([128, SLAB, 3 * H], bf16, tag="xs")
                    nc.sync.dma_start(out=xs_cur, in_=xgd[:, t0:t0 + SLAB, :])

                def hlhs(c, tau):
                    if ell == 1:
                        return ybf[:, c, tau, :]
                    return hbf2[:, c, :]

                def prefill(tau):
                    """bias + input-side gates for tick tau (psum dbl-buffered).

                    Layer 1: host-precomputed xg row-block is injected into
                    PSUM via one identity matmul per gate (out = I.T @ xg);
                    biases are folded into xg on the host.
                    Layer 2: bias matmul + y-side GEMM as usual."""
                    ps_r = pra.tile([128, GW], f32, tag="ps_r")
                    ps_z = pzb.tile([128, GW], f32, tag="ps_z")
                    ps_xn = pxc.tile([128, GW], f32, tag="ps_xn")
                    if ell == 1:
                        for gi, p in ((0, ps_r), (1, ps_z), (2, ps_xn)):
                            nc.tensor.matmul(
                                p, ident_sb,
                                xs_cur[:, tau % SLAB, gi * GW:(gi + 1) * GW],
                                start=True, stop=(gi == 2))
                        return [ps_r, ps_z, ps_xn]
                    for gi, p in ((0, ps_r), (1, ps_z), (2, ps_xn)):
                        nc.tensor.matmul(p, ones_sb[0:1, :], bi[0:1, gi, :],
                                         start=True, stop=False)
                    for c in range(C):
                        lx = ybf[:, c, W1 + 1 + tau, :]
                        nc.tensor.matmul(ps_r, lx, wi[:, c, 0:GW],
                                         start=False, stop=False)
                        nc.tensor.matmul(ps_z, lx, wi[:, c, GW:2 * GW],
                                         start=False, stop=False)
                        nc.tensor.matmul(ps_xn, lx, wi[:, c, 2 * GW:3 * GW],
                                         start=False, stop=(c == C - 1))
                    return [ps_r, ps_z, ps_xn]

                for tau in range(TK):
                    if tau == 0:
                        if ell == 1:
                            load_slab(0)
                        ps[0] = prefill(0)
                    ps_r, ps_z, ps_xn = ps[tau % 2]

                    # hidden-side: r first (EW starts early), then hn, then z
                    for c in range(C):
                        nc.tensor.matmul(ps_r, hlhs(c, tau), wh[:, c, 0:GW],
                                         start=False, stop=(c == C - 1))
                    ps_hn = phnp.tile([128, GW], f32, tag="ps_hn")
                    nc.tensor.matmul(ps_hn, ones_sb[0:1, :], bi[0:1, 3, :],
                                     start=True, stop=False)
                    for c in range(C):
                        nc.tensor.matmul(ps_hn, hlhs(c, tau), wh[:, c, 2 * GW:3 * GW],
                                         start=False, stop=(c == C - 1))
                    for c in range(C):
                        nc.tensor.matmul(ps_z, hlhs(c, tau), wh[:, c, GW:2 * GW],
                                         start=False, stop=(c == C - 1))

                    # prefill next tick: runs on PE while DVE/ACT do this tick's EW
                    if tau + 1 < TK:
                        if ell == 1 and (tau + 1) % SLAB == 0:
                            load_slab(tau + 1)
                        ps[(tau + 1) % 2] = prefill(tau + 1)

                    # ---- elementwise ([rows, hid] orientation) ----
                    r_t = ew.tile([128, GW], bf16, tag="r")
                    z_t = ew.tile([128, GW], bf16, tag="z")
                    v_t = ew.tile([128, GW], bf16, tag="v")
                    np_t = ew.tile([128, GW], f32, tag="npre")
                    n_t = ew.tile([128, GW], bf16, tag="n")
                    d_t = ew.tile([128, GW], bf16, tag="d")
                    e_t = ew.tile([128, GW], bf16, tag="e")
                    hrow = ew.tile([128, GW], bf16, tag="hrow")
                    nc.scalar.activation(r_t, ps_r, SIG)
                    nc.vector.tensor_mul(v_t, ps_hn, r_t)       # r*(hn+b_hn)
                    nc.vector.tensor_add(np_t, ps_xn, v_t)
                    nc.scalar.activation(n_t, np_t, TANH)
                    nc.scalar.activation(z_t, ps_z, SIG)
                    nc.vector.tensor_sub(d_t, h32, n_t)
                    nc.vector.tensor_mul(e_t, z_t, d_t)
                    nc.vector.tensor_add(h32, n_t, e_t)         # h' = n + z*(h-n)
                    if tau == WM - 1:
                        # zero rows of stream 0 on core 0 (true h at t=0 is 0)
                        nc.vector.tensor_scalar_mul(h32, h32, mask_sb[:, 0:1])
                    nc.scalar.copy(hrow, h32)                   # bf16 cast

                    # transpose back to [hid, rows]; all 4 chunks in one bank
                    pT = ptr.tile([128, C, R], bf16, tag="pT")
                    for c in range(C):
                        nc.tensor.matmul(pT[:, c, :], hrow[:, c * 128:(c + 1) * 128],
                                         ident_sb, is_transpose=True,
                                         start=(c == 0), stop=(c == C - 1))
                    if ell == 1:
                        nc.scalar.copy(ybf[:, :, tau + 1, :], pT[:, :, :])
                    else:
                        nc.scalar.copy(hbf2, pT[:, :, :])
                        if tau >= W2:
                            ot = outp.tile([128, C, R], f32, tag="ot")
                            nc.vector.tensor_add(ot, pT[:, :, :],
                                                 ybf[:, :, W1 + 1 + tau, :])
                            nc.sync.dma_start(out=od[:, :, tau - W2, :], in_=ot)
    return nc


def _legalize_waits(nc):
    """Hardware instruction encodings hold a limited number of sync waits
    (core_v3 Matmult: 1, DVE STT and friends: 2).  Spill excess waits onto
    same-engine NoOps inserted immediately before the instruction."""
    import bass_rust
    from concourse import mybir

    caps = {}
    nop_cap = 1
    moved = 0
    uid = [0]
    for blk in nc.m.functions[0].blocks:
        idx = 0
        while idx < len(blk.instructions):
            ins = blk.instructions[idx]
            ty = type(ins).__name__
            if ty in ("InstNoOp", "InstEventSemaphore",
                      "InstUnconditionalBranch", "InstCall", "InstISA"):
                idx += 1
                continue
            si = ins.sync_info
            if si is None:
                idx += 1
                continue
            cap = caps.get(ty, 1)
            waits = list(si.on_wait)
            if len(waits) <= cap:
                idx += 1
                continue
            excess = waits[:-cap] if cap else waits
            keep = waits[-cap:] if cap else []
            nops = []
            while excess:
                chunk, excess = excess[:nop_cap], excess[nop_cap:]
                uid[0] += 1
                nop = mybir.InstNoOp(name=f"waitnop-{uid[0]}", ins=[], outs=[])
                nop.engine = ins.engine
                nop.sync_info = bass_rust.SyncInfo(on_wait=chunk, on_update=[])
                nops.append(nop)
                moved += len(chunk)
            for k, nop in enumerate(nops):
                blk.instructions.insert(idx + k, nop)
            ins2 = blk.instructions[idx + len(nops)]
            assert ins2.name == ins.name
            si.on_wait = keep
            ins2.sync_info = si
            idx += len(nops) + 1
    return moved


def _prep_inputs(x, W_ih1, W_hh1, b_ih1, b_hh1, W_ih2, W_hh2, b_ih2, b_hh2):
    bf = ml_dtypes.bfloat16
    f32 = np.float32
    WP = W1 + W2  # x prefix ticks

    def wT(Wm):  # [3H, H] -> [128, C, 3H] tiles: [p, c, g] = Wm[g, c*128+p]
        return np.ascontiguousarray(
            Wm.T.reshape(C, 128, 3 * H).transpose(1, 0, 2)).astype(bf)

    def biasrows(bi, bh):  # per-gate bias rows on partition 0: r, z, xn, hn
        out = np.zeros((128, 4, GW), np.float32)
        s = bi + bh
        out[0, 0, :] = s[:H]
        out[0, 1, :] = s[H:2 * H]
        out[0, 2, :] = bi[2 * H:]
        out[0, 3, :] = bh[2 * H:]
        return out.astype(bf)

    ones = np.zeros((128, 128), np.float32)
    ones[0, :] = 1.0
    ident = np.eye(128, dtype=np.float32)
    common = {
        "whh1": wT(W_hh1),
        "wih2": wT(W_ih2), "whh2": wT(W_hh2),
        "bias1": biasrows(b_ih1, b_hh1), "bias2": biasrows(b_ih2, b_hh2),
        "onesd": ones.astype(bf), "identd": ident.astype(bf),
    }

    # layer-1 input gates on host: f32 BLAS GEMM, biases folded
    # (r,z get b_ih+b_hh; xn gets b_ih only — b_hh_n rides in the hn psum)
    bfold = np.concatenate([(b_ih1 + b_hh1)[:2 * H], b_ih1[2 * H:]]).astype(np.float32)
    xg = x.reshape(-1, IN).astype(np.float32) @ W_ih1.T.astype(np.float32)
    xg = (xg.reshape(B, T, 3 * H) + bfold).astype(np.float32)
    # x is zero-padded for t<0, so xg there is just the bias row
    xgpad = np.concatenate(
        [np.broadcast_to(bfold, (B, WP, 3 * H)), xg], axis=1)  # [B, T+WP, 3H]
    in_maps = []
    for p in range(NCORES):
        segs = np.stack([xgpad[:, (p * S + s) * L: (p * S + s) * L + TK1, :]
                         for s in range(S)])              # [S, B, TK1, 3H]
        xgp = segs.transpose(0, 1, 2, 3).reshape(S * B, TK1, 3 * H).astype(bf)
        mask = np.ones((128, 1), np.float32)
        if p == 0:
            mask[0:B, 0] = 0.0  # rows of stream 0 (true h at chunk start is 0)
        in_maps.append({"xgd": np.ascontiguousarray(xgp),
                        "maskd": mask, **common})
    return in_maps


def _postprocess(results):
    out = np.empty((B, T, H), np.float32)
    for p in range(NCORES):
        o = results[p]["od"]                    # [128, C, L, R]
        o = o.reshape(128, C, L, S, B).transpose(4, 3, 2, 1, 0) \
             .reshape(B, S * L, H)
        out[:, p * S * L:(p + 1) * S * L, :] = o
    return out


def kernel(**inputs):
    from concourse.bass_utils import run_bass_kernel_spmd

    if "nc" not in _cache:
        nc = _build_bass()
        _legalize_waits(nc)
        _cache["nc"] = nc
    nc = _cache["nc"]
    in_maps = _prep_inputs(**inputs)
    res = run_bass_kernel_spmd(nc, in_maps, core_ids=list(range(NCORES)))
    return _postprocess(res.results)


# revision 6
# speedup vs baseline: 1.8439x; 1.8157x over previous
"""Two-layer GRU + residual on 8 Trainium2 NeuronCores — v2.

Strategy (v2): sequence-chunked streams as in v1, but with
  * h-stationary matmuls: per contraction chunk c, lhsT = h[:, c, :]
    ([hid-c, rows]) is the stationary operand and the rhs is a 512-wide
    slice of the weight matrix, so each gate bank needs 4 matmuls of
    N=512 instead of 16 of N=128.  Gate psums come out [rows, gates].
  * per-tick PE transpose of the new state back to [hid, rows] (4
    transposes of 128x128 into one PSUM bank), which simultaneously
    produces the bf16 y-history slot used by layer 2.
  * two-level warmup: layer 1 warms up W1 ticks and emits W2+L outputs;
    layer 2 consumes the first W2 as its own warmup.  W1=W2=16 (state
    influence decays ~0.55/step; 16 steps => ~2e-3, well under the 2e-2
    gate).  v1 used W=64.
  * y history lives entirely in SBUF (no DRAM roundtrip between layers).
Ticks: 96 (L1) + 80 (L2) vs v1's 256, with ~3x fewer PE instructions
per tick.
"""

import sys
import numpy as np
import ml_dtypes

sys.path.insert(0, "/opt/trn_rl_repo")

# ---- problem constants (hardcoded per contract) ----
B, T, IN, H = 16, 4096, 512, 512
NCORES = 8
S = 8             # streams (time chunks) per core
R = S * B         # 128 rows per core
L = 64            # useful chunk length; NCORES*S*L == T
W1 = 16           # layer-1 warmup ticks
W2 = 16           # layer-2 warmup ticks
TK1 = W1 + W2 + L # 96 layer-1 ticks
TK2 = W2 + L      # 80 layer-2 ticks
C = 4             # hidden chunks of 128 (H/128)
GW = 512          # gate width per psum bank (H)
SLAB = 8          # ticks per input DMA slab

_cache = {}


def _build_bass():
    import concourse.bass as bass
    import concourse.tile as tile
    from concourse import mybir

    f32 = mybir.dt.float32
    bf16 = mybir.dt.bfloat16
    SIG = mybir.ActivationFunctionType.Sigmoid
    TANH = mybir.ActivationFunctionType.Tanh
    COPY = mybir.ActivationFunctionType.Copy

    nc = bass.Bass("TRN2")

    xd = nc.dram_tensor("xd", [128, C, TK1, R], bf16, kind="ExternalInput")
    wih = [None, nc.dram_tensor("wih1", [128, C, 3 * H], bf16, kind="ExternalInput"),
           nc.dram_tensor("wih2", [128, C, 3 * H], bf16, kind="ExternalInput")]
    whh = [None, nc.dram_tensor("whh1", [128, C, 3 * H], bf16, kind="ExternalInput"),
           nc.dram_tensor("whh2", [128, C, 3 * H], bf16, kind="ExternalInput")]
    # per-gate bias rows (r, z, xn, hn) on partition 0, each GW wide
    biasd = [None, nc.dram_tensor("bias1", [128, 4, GW], bf16, kind="ExternalInput"),
             nc.dram_tensor("bias2", [128, 4, GW], bf16, kind="ExternalInput")]
    onesd = nc.dram_tensor("onesd", [128, 128], bf16, kind="ExternalInput")
    identd = nc.dram_tensor("identd", [128, 128], bf16, kind="ExternalInput")
    maskd = nc.dram_tensor("maskd", [128, 1], f32, kind="ExternalInput")
    od = nc.dram_tensor("od", [128, C, L, R], f32, kind="ExternalOutput")

    with tile.TileContext(nc) as tc:
        with (
            tc.tile_pool(name="const", bufs=1) as const,
            tc.tile_pool(name="state", bufs=1) as state,
            tc.tile_pool(name="xslab", bufs=2) as xslab,
            tc.tile_pool(name="ew", bufs=2) as ew,
            tc.tile_pool(name="outp", bufs=3) as outp,
            tc.tile_pool(name="pra", bufs=2, space="PSUM") as pra,
            tc.tile_pool(name="pzb", bufs=2, space="PSUM") as pzb,
            tc.tile_pool(name="pxc", bufs=2, space="PSUM") as pxc,
            tc.tile_pool(name="phn", bufs=1, space="PSUM") as phnp,
            tc.tile_pool(name="ptr", bufs=1, space="PSUM") as ptr,
        ):
            # ---- constants to SBUF ----
            wih_sb, whh_sb, bias_sb = {}, {}, {}
            for ell in (1, 2):
                wih_sb[ell] = const.tile([128, C, 3 * H], bf16, tag=f"wih{ell}", name=f"wih_sb{ell}")
                nc.sync.dma_start(out=wih_sb[ell], in_=wih[ell][:])
                whh_sb[ell] = const.tile([128, C, 3 * H], bf16, tag=f"whh{ell}", name=f"whh_sb{ell}")
                nc.sync.dma_start(out=whh_sb[ell], in_=whh[ell][:])
                bias_sb[ell] = const.tile([128, 4, GW], bf16, tag=f"bias{ell}", name=f"bias_sb{ell}")
                nc.sync.dma_start(out=bias_sb[ell], in_=biasd[ell][:])
            ones_sb = const.tile([128, 128], bf16)
            nc.sync.dma_start(out=ones_sb, in_=onesd[:])
            ident_sb = const.tile([128, 128], bf16)
            nc.sync.dma_start(out=ident_sb, in_=identd[:])
            mask_sb = const.tile([128, 1], f32)
            nc.sync.dma_start(out=mask_sb, in_=maskd[:])

            # y history: slot k = h1 state BEFORE layer-1 tick k (slot 0 = 0)
            ybf = const.tile([128, C, TK1 + 1, R], bf16, name="ybf")
            h32 = state.tile([128, GW], f32)       # [rows, hid] fp32 state
            hbf2 = state.tile([128, C, R], bf16)   # layer-2 [hid, rows] state

            for ell in (1, 2):
                wi, wh, bi = wih_sb[ell], whh_sb[ell], bias_sb[ell]
                TK = TK1 if ell == 1 else TK2
                WM = W1 + W2 if ell == 1 else W2   # mask tick boundary
                nc.vector.memset(h32, 0.0)
                if ell == 1:
                    nc.vector.memset(ybf[:, :, 0, :], 0.0)
                else:
                    nc.vector.memset(hbf2, 0.0)

                xs_cur = None
                ps = [None, None]

                def load_slab(t0):
                    nonlocal xs_cur
                    xs_cur = xslab.tile([128, C, SLAB, R], bf16, tag="xs")
                    nc.sync.dma_start(out=xs_cur, in_=xd[:, :, t0:t0 + SLAB, :])

                def xlhs(c, tau):
                    if ell == 1:
                        return xs_cur[:, c, tau % SLAB, :]
                    return ybf[:, c, W1 + 1 + tau, :]

                def hlhs(c, tau):
                    if ell == 1:
                        return ybf[:, c, tau, :]
                    return hbf2[:, c, :]

                def prefill(tau):
                    """bias + input-side matmuls for tick tau (psum dbl-buffered)."""
                    ps_r = pra.tile([128, GW], f32, tag="ps_r")
                    ps_z = pzb.tile([128, GW], f32, tag="ps_z")
                    ps_xn = pxc.tile([128, GW], f32, tag="ps_xn")
                    for gi, p in ((0, ps_r), (1, ps_z), (2, ps_xn)):
                        nc.tensor.matmul(p, ones_sb[0:1, :], bi[0:1, gi, :],
                                         start=True, stop=False)
                    for c in range(C):
                        lx = xlhs(c, tau)
                        nc.tensor.matmul(ps_r, lx, wi[:, c, 0:GW],
                                         start=False, stop=False)
                        nc.tensor.matmul(ps_z, lx, wi[:, c, GW:2 * GW],
                                         start=False, stop=False)
                        nc.tensor.matmul(ps_xn, lx, wi[:, c, 2 * GW:3 * GW],
                                         start=False, stop=(c == C - 1))
                    return [ps_r, ps_z, ps_xn]

                for tau in range(TK):
                    if tau == 0:
                        if ell == 1:
                            load_slab(0)
                        ps[0] = prefill(0)
                    ps_r, ps_z, ps_xn = ps[tau % 2]

                    # hidden-side: r first (EW starts early), then hn, then z
                    for c in range(C):
                        nc.tensor.matmul(ps_r, hlhs(c, tau), wh[:, c, 0:GW],
                                         start=False, stop=(c == C - 1))
                    ps_hn = phnp.tile([128, GW], f32, tag="ps_hn")
                    nc.tensor.matmul(ps_hn, ones_sb[0:1, :], bi[0:1, 3, :],
                                     start=True, stop=False)
                    for c in range(C):
                        nc.tensor.matmul(ps_hn, hlhs(c, tau), wh[:, c, 2 * GW:3 * GW],
                                         start=False, stop=(c == C - 1))
                    for c in range(C):
                        nc.tensor.matmul(ps_z, hlhs(c, tau), wh[:, c, GW:2 * GW],
                                         start=False, stop=(c == C - 1))

                    # prefill next tick: runs on PE while DVE/ACT do this tick's EW
                    if tau + 1 < TK:
                        if ell == 1 and (tau + 1) % SLAB == 0:
                            load_slab(tau + 1)
                        ps[(tau + 1) % 2] = prefill(tau + 1)

                    # ---- elementwise ([rows, hid] orientation) ----
                    r_t = ew.tile([128, GW], bf16, tag="r")
                    z_t = ew.tile([128, GW], bf16, tag="z")
                    v_t = ew.tile([128, GW], bf16, tag="v")
                    np_t = ew.tile([128, GW], f32, tag="npre")
                    n_t = ew.tile([128, GW], bf16, tag="n")
                    d_t = ew.tile([128, GW], bf16, tag="d")
                    e_t = ew.tile([128, GW], bf16, tag="e")
                    hrow = ew.tile([128, GW], bf16, tag="hrow")
                    nc.scalar.activation(r_t, ps_r, SIG)
                    nc.vector.tensor_mul(v_t, ps_hn, r_t)       # r*(hn+b_hn)
                    nc.vector.tensor_add(np_t, ps_xn, v_t)
                    nc.scalar.activation(n_t, np_t, TANH)
                    nc.scalar.activation(z_t, ps_z, SIG)
                    nc.vector.tensor_sub(d_t, h32, n_t)
                    nc.vector.tensor_mul(e_t, z_t, d_t)
                    nc.vector.tensor_add(h32, n_t, e_t)         # h' = n + z*(h-n)
                    if tau == WM - 1:
                        # zero rows of stream 0 on core 0 (true h at t=0 is 0)
                        nc.vector.tensor_scalar_mul(h32, h32, mask_sb[:, 0:1])
                    nc.scalar.copy(hrow, h32)                   # bf16 cast

                    # transpose back to [hid, rows]; all 4 chunks in one bank
                    pT = ptr.tile([128, C, R], bf16, tag="pT")
                    for c in range(C):
                        nc.tensor.matmul(pT[:, c, :], hrow[:, c * 128:(c + 1) * 128],
                                         ident_sb, is_transpose=True,
                                         start=(c == 0), stop=(c == C - 1))
                    if ell == 1:
                        nc.scalar.copy(ybf[:, :, tau + 1, :], pT[:, :, :])
                    else:
                        nc.scalar.copy(hbf2, pT[:, :, :])
                        if tau >= W2:
                            ot = outp.tile([128, C, R], f32, tag="ot")
                            nc.vector.tensor_add(ot, pT[:, :, :],
                                                 ybf[:, :, W1 + 1 + tau, :])
                            nc.sync.dma_start(out=od[:, :, tau - W2, :], in_=ot)
    return nc


def _legalize_waits(nc):
    """Hardware instruction encodings hold a limited number of sync waits
    (core_v3 Matmult: 1, DVE STT and friends: 2).  Spill excess waits onto
    same-engine NoOps inserted immediately before the instruction."""
    import bass_rust
    from concourse import mybir

    caps = {}
    nop_cap = 1
    moved = 0
    uid = [0]
    for blk in nc.m.functions[0].blocks:
        idx = 0
        while idx < len(blk.instructions):
            ins = blk.instructions[idx]
            ty = type(ins).__name__
            if ty in ("InstNoOp", "InstEventSemaphore",
                      "InstUnconditionalBranch", "InstCall", "InstISA"):
                idx += 1
                continue
            si = ins.sync_info
            if si is None:
                idx += 1
                continue
            cap = caps.get(ty, 1)
            waits = list(si.on_wait)
            if len(waits) <= cap:
                idx += 1
                continue
            excess = waits[:-cap] if cap else waits
            keep = waits[-cap:] if cap else []
            nops = []
            while excess:
                chunk, excess = excess[:nop_cap], excess[nop_cap:]
                uid[0] += 1
                nop = mybir.InstNoOp(name=f"waitnop-{uid[0]}", ins=[], outs=[])
                nop.engine = ins.engine
                nop.sync_info = bass_rust.SyncInfo(on_wait=chunk, on_update=[])
                nops.append(nop)
                moved += len(chunk)
            for k, nop in enumerate(nops):
                blk.instructions.insert(idx + k, nop)
            ins2 = blk.instructions[idx + len(nops)]
            assert ins2.name == ins.name
            si.on_wait = keep
            ins2.sync_info = si
            idx += len(nops) + 1
    return moved


def _prep_inputs(x, W_ih1, W_hh1, b_ih1, b_hh1, W_ih2, W_hh2, b_ih2, b_hh2):
    bf = ml_dtypes.bfloat16
    f32 = np.float32
    WP = W1 + W2  # x prefix ticks

    def wT(Wm):  # [3H, H] -> [128, C, 3H] tiles: [p, c, g] = Wm[g, c*128+p]
        return np.ascontiguousarray(
            Wm.T.reshape(C, 128, 3 * H).transpose(1, 0, 2)).astype(bf)

    def biasrows(bi, bh):  # per-gate bias rows on partition 0: r, z, xn, hn
        out = np.zeros((128, 4, GW), np.float32)
        s = bi + bh
        out[0, 0, :] = s[:H]
        out[0, 1, :] = s[H:2 * H]
        out[0, 2, :] = bi[2 * H:]
        out[0, 3, :] = bh[2 * H:]
        return out.astype(bf)

    ones = np.zeros((128, 128), np.float32)
    ones[0, :] = 1.0
    ident = np.eye(128, dtype=np.float32)
    common = {
        "wih1": wT(W_ih1), "whh1": wT(W_hh1),
        "wih2": wT(W_ih2), "whh2": wT(W_hh2),
        "bias1": biasrows(b_ih1, b_hh1), "bias2": biasrows(b_ih2, b_hh2),
        "onesd": ones.astype(bf), "identd": ident.astype(bf),
    }

    # x -> per-core [128, C, TK1, R] bf16 with WP ticks of (zero-padded) history
    xpad = np.concatenate([np.zeros((B, WP, IN), np.float32), x], axis=1)
    in_maps = []
    for p in range(NCORES):
        segs = np.stack([xpad[:, (p * S + s) * L: (p * S + s) * L + TK1, :]
                         for s in range(S)])              # [S, B, TK1, IN]
        xdp = segs.reshape(S, B, TK1, C, 128).transpose(4, 3, 2, 0, 1) \
                  .reshape(128, C, TK1, R).astype(bf)
        mask = np.ones((128, 1), np.float32)
        if p == 0:
            mask[0:B, 0] = 0.0  # rows of stream 0 (true h at chunk start is 0)
        in_maps.append({"xd": np.ascontiguousarray(xdp),
                        "maskd": mask, **common})
    return in_maps


def _postprocess(results):
    out = np.empty((B, T, H), np.float32)
    for p in range(NCORES):
        o = results[p]["od"]                    # [128, C, L, R]
        o = o.reshape(128, C, L, S, B).transpose(4, 3, 2, 1, 0) \
             .reshape(B, S * L, H)
        out[:, p * S * L:(p + 1) * S * L, :] = o
    return out


def kernel(**inputs):
    from concourse.bass_utils import run_bass_kernel_spmd

    if "nc" not in _cache:
        nc = _build_bass()
        _legalize_waits(nc)
        _cache["nc"] = nc
    nc = _cache["nc"]
    in_maps = _prep_inputs(**inputs)
    res = run_bass_kernel_spmd(nc, in_maps, core_ids=list(range(NCORES)))
    return _postprocess(res.results)
